# revision 11
# baseline (speedup 1.0000x reference)
"""Trainium2 Bass kernel for nn_MultiHeadAttention_62319975465542.

Tensor-parallel over heads (Megatron-style): 32 heads sharded 4-per-core
across 8 NeuronCores. Each core computes its heads' QKV projections,
attention, and a partial output projection; the host sums the 8 partials
(the all-reduce after Wo) and adds the output bias.

Reference layout note: Q = (X @ Wq.T + b).reshape(L, D_HEAD, NUM_HEADS),
so head h owns interleaved feature columns {d*32 + h : d in 0..63}. The
host pre-gathers those columns into contiguous per-core blocks.

All matmuls run as float32r (TF32-like, 1 cycle/row for N>=256); every
tensor feeding a matmul is declared float32r end-to-end (the compiler
requires producers to round to f32r). The numpy byte layout is float32.
"""

import numpy as np

import concourse.bass as bass
import concourse.tile as tile
import concourse.mybir as mybir
from concourse import bacc

F32 = mybir.dt.float32
F32R = mybir.dt.float32r
BF16 = mybir.dt.bfloat16
Identity = mybir.ActivationFunctionType.Identity
Exp = mybir.ActivationFunctionType.Exp

L = 2048          # sequence length
D = 2048          # d_model
NH = 32           # total heads
DH = 64           # head dim
NCORES = 8
HPC = NH // NCORES   # heads per core = 4
JC = HPC * DH        # per-core projected width = 256
LB = 512             # l-block width
NLB = L // LB        # 4
KO = D // 128        # 16 contraction chunks
MC = L // 128        # 16 key chunks


def build_program():
    nc = bacc.Bacc("TRN2", target_bir_lowering=False, debug=False)

    xt_d = nc.dram_tensor("XT", (D, L), F32R, kind="ExternalInput")
    wq_d = nc.dram_tensor("WQ", (128, KO, JC), F32R, kind="ExternalInput")
    wk_d = nc.dram_tensor("WK", (128, KO, JC), F32R, kind="ExternalInput")
    wv_d = nc.dram_tensor("WV", (128, KO, JC), F32R, kind="ExternalInput")
    wo_d = nc.dram_tensor("WO", (128, 2, D), F32R, kind="ExternalInput")
    bq_d = nc.dram_tensor("BQ", (128, 2), F32, kind="ExternalInput")
    bk_d = nc.dram_tensor("BK", (128, 2), F32, kind="ExternalInput")
    bv_d = nc.dram_tensor("BV", (1, JC), F32R, kind="ExternalInput")
    onesr_d = nc.dram_tensor("ONESR", (1, 128), F32R, kind="ExternalInput")
    y_d = nc.dram_tensor("Y", (L, D), F32, kind="ExternalOutput")

    with tile.TileContext(nc) as tc, nc.allow_low_precision(
            reason="float32r outputs are fp32-width; rounding is intended"):
        with tc.tile_pool(name="const", bufs=1) as cp:
            wq_sb = cp.tile((128, KO, JC), F32R)
            wk_sb = cp.tile((128, KO, JC), F32R)
            wv_sb = cp.tile((128, KO, JC), F32R)
            wo_sb = cp.tile((128, 2, D), F32R)
            bq_sb = cp.tile((128, 2), F32)
            bk_sb = cp.tile((128, 2), F32)
            bv_sb = cp.tile((1, JC), F32R)
            ones_sb = cp.tile((128, 128), BF16)
            onesr_sb = cp.tile((1, 128), F32R)
            nc.sync.dma_start(wq_sb[:], wq_d[:])
            nc.sync.dma_start(wk_sb[:], wk_d[:])
            nc.sync.dma_start(wv_sb[:], wv_d[:])
            nc.sync.dma_start(wo_sb[:], wo_d[:])
            nc.sync.dma_start(bq_sb[:], bq_d[:])
            nc.sync.dma_start(bk_sb[:], bk_d[:])
            nc.sync.dma_start(bv_sb[:], bv_d[:])
            nc.sync.dma_start(onesr_sb[:], onesr_d[:])
            nc.vector.memset(ones_sb[:], 1.0)

            # persistent activations
            qt_sb = [cp.tile((128, L), F32R, name=f"qt{p}") for p in range(2)]
            kt_sb = [cp.tile((128, L), F32R, name=f"kt{p}") for p in range(2)]
            v_sb = cp.tile((128, MC, JC), BF16)
            ot_sb = [cp.tile((128, L), F32R, name=f"ot{p}") for p in range(2)]

            # ---------------- Phase 1: QKV projections ----------------
            with (
                tc.tile_pool(name="xt", bufs=2) as xtp,
                tc.tile_pool(name="qkps", bufs=4, space="PSUM") as qkps,
                tc.tile_pool(name="vps", bufs=4, space="PSUM") as vps,
            ):
                for lb in range(NLB):
                    xt = xtp.tile((128, KO, LB), F32R, name="xt_t")
                    for ko in range(KO):
                        nc.sync.dma_start(
                            xt[:, ko, :],
                            xt_d[ko * 128:(ko + 1) * 128, lb * LB:(lb + 1) * LB],
                        )
                    # K^T then Q^T (both (j, l) layout, bias per partition j)
                    for w_sb, b_sb, dst in ((wk_sb, bk_sb, kt_sb),
                                            (wq_sb, bq_sb, qt_sb)):
                        for jc in range(2):
                            ps = qkps.tile((128, LB), F32, name="qk_ps")
                            for ko in range(KO):
                                nc.tensor.matmul(
                                    ps[:],
                                    (w_sb[:, ko, jc * 128:(jc + 1) * 128]),
                                    (xt[:, ko, :]),
                                    start=(ko == 0), stop=(ko == KO - 1),
                                )
                            nc.scalar.activation(
                                dst[jc][:, lb * LB:(lb + 1) * LB], ps[:],
                                Identity, bias=b_sb[:, jc:jc + 1],
                            )
                    # V in (l, j) layout; bias via K=1 ones-row matmul
                    for lt in range(4):
                        vp = vps.tile((128, JC), F32, name="v_ps")
                        for ko in range(KO):
                            nc.tensor.matmul(
                                vp[:],
                                (xt[:, ko, lt * 128:(lt + 1) * 128]),
                                (wv_sb[:, ko, :]),
                                start=(ko == 0), stop=False,
                            )
                        nc.tensor.matmul(
                            vp[:], (onesr_sb[0:1, :]), (bv_sb[0:1, :]),
                            start=False, stop=True,
                        )
                        nc.vector.tensor_copy(v_sb[:, lb * 4 + lt, :], vp[:])

            # ---------------- Phase 2: attention + out-projection ----------------
            with (
                tc.tile_pool(name="epool", bufs=3) as epool,
                tc.tile_pool(name="norm", bufs=2) as normp,
                tc.tile_pool(name="ysb", bufs=3) as ypool,
                tc.tile_pool(name="scps", bufs=2, space="PSUM") as scps,
                tc.tile_pool(name="accps", bufs=3, space="PSUM") as accps,
                tc.tile_pool(name="denps", bufs=1, space="PSUM") as denps,
            ):
                for lb in range(NLB):
                    lsl = slice(lb * LB, (lb + 1) * LB)
                    for p in range(2):  # head pair (local heads 2p, 2p+1)
                        av = accps.tile((128, LB), F32, name="acc_ps")
                        den = denps.tile((128, LB), F32, name="den_ps")
                        for m in range(MC):
                            msl = slice(m * 128, (m + 1) * 128)
                            sc = scps.tile((128, 2 * LB), F32, name="sc_ps")
                            # S^T = K·Q^T per head; heads at partition 0:64/64:128
                            nc.tensor.matmul(
                                sc[:, 0:LB],
                                (kt_sb[p][0:64, msl]), (qt_sb[p][0:64, lsl]),
                            )
                            nc.tensor.matmul(
                                sc[:, LB:2 * LB],
                                (kt_sb[p][64:128, msl]), (qt_sb[p][64:128, lsl]),
                            )
                            e = epool.tile((128, 2 * LB), BF16, name="e_sb")
                            nc.scalar.activation(e[:], sc[:], Exp)
                            # A·V col-packed: head 2p -> rows 0:64, 2p+1 -> 64:128.
                            # start clears pending-zero bits per partition (only
                            # the matmul's own partitions), so each partition
                            # range is its own accumulation group. The sim's
                            # coarse group CHECK is partition-blind, hence
                            # skip_group_check.
                            nc.tensor.matmul(
                                av[0:64, :],
                                (v_sb[:, m, (2 * p) * DH:(2 * p + 1) * DH]),
                                (e[:, 0:LB]),
                                start=(m == 0), stop=(m == MC - 1),
                                skip_group_check=True,
                            )
                            nc.tensor.matmul(
                                av[64:128, :],
                                (v_sb[:, m, (2 * p + 1) * DH:(2 * p + 2) * DH]),
                                (e[:, LB:2 * LB]),
                                start=(m == 0), stop=(m == MC - 1),
                                skip_group_check=True,
                            )
                            # denominators: M=1 ones-matmuls at rows 0 and 32
                            nc.tensor.matmul(
                                den[0:1, :], (ones_sb[:, 0:1]), (e[:, 0:LB]),
                                start=(m == 0), stop=(m == MC - 1),
                                skip_group_check=True,
                            )
                            nc.tensor.matmul(
                                den[32:33, :], (ones_sb[:, 0:1]),
                                (e[:, LB:2 * LB]),
                                start=(m == 0), stop=(m == MC - 1),
                                skip_group_check=True,
                            )
                        den_sb = normp.tile((33, LB), BF16, name="den_sb")
                        nc.vector.reciprocal(den_sb[0:1, :], den[0:1, :])
                        nc.vector.reciprocal(den_sb[32:33, :], den[32:33, :])
                        rb = denps.tile((128, LB), F32, name="den_ps")
                        nc.tensor.matmul(
                            rb[0:64, :], (ones_sb[0:1, 0:64]), (den_sb[0:1, :]),
                            skip_group_check=True,
                        )
                        nc.tensor.matmul(
                            rb[64:128, :], (ones_sb[32:33, 0:64]),
                            (den_sb[32:33, :]),
                            skip_group_check=True,
                        )
                        rb_sb = normp.tile((128, LB), F32, name="rb_sb")
                        nc.vector.tensor_copy(rb_sb[:], rb[:])
                        nc.vector.tensor_tensor(
                            ot_sb[p][:, lsl], av[:], rb_sb[:],
                            mybir.AluOpType.mult,
                        )
                    # out-projection for this l-block
                    for lt in range(4):
                        row0 = lb * LB + lt * 128
                        for ns in range(4):
                            yp = accps.tile((128, 512), F32, name="acc_ps")
                            for jc in range(2):
                                nc.tensor.matmul(
                                    yp[:],
                                    (ot_sb[jc][:, row0:row0 + 128]),
                                    (wo_sb[:, jc, ns * 512:(ns + 1) * 512]),
                                    start=(jc == 0), stop=(jc == 1),
                                )
                            ty = ypool.tile((128, 512), F32, name="y_sb")
                            nc.any.tensor_copy(ty[:], yp[:])
                            nc.sync.dma_start(
                                y_d[row0:row0 + 128, ns * 512:(ns + 1) * 512],
                                ty[:],
                            )

    nc.compile()
    return nc


def make_core_inputs(X, Wq_w, Wq_b, Wk_w, Wk_b, Wv_w, Wv_b, Wo_w):
    """Host-side sharding: per-core input dicts (shared XT + per-core weights)."""
    X = np.asarray(X, np.float32)
    xt = np.ascontiguousarray(X.T)
    scale = 1.0 / np.sqrt(np.float32(D))
    in_maps = []
    for c in range(NCORES):
        idx = np.array([d * NH + h for h in range(c * HPC, (c + 1) * HPC)
                        for d in range(DH)], np.int64)

        def kxj(w, s=1.0):
            # (D_in=K, JC) -> (128, KO, JC) with [p, ko, j] = w.T[ko*128+p, j]
            wt = np.ascontiguousarray((np.asarray(w, np.float32)[idx, :] * s).T)
            return np.ascontiguousarray(wt.reshape(KO, 128, JC).transpose(1, 0, 2))

        wo = np.ascontiguousarray(np.asarray(Wo_w, np.float32)[:, idx].T)  # (JC, D)
        wo = np.ascontiguousarray(wo.reshape(2, 128, D).transpose(1, 0, 2))

        def bcol(b, s=1.0):
            return np.ascontiguousarray(
                (np.asarray(b, np.float32)[idx] * s).reshape(2, 128).T)

        in_maps.append({
            "XT": xt,
            "WQ": kxj(Wq_w, scale), "WK": kxj(Wk_w), "WV": kxj(Wv_w),
            "WO": wo,
            "BQ": bcol(Wq_b, scale), "BK": bcol(Wk_b),
            "BV": np.ascontiguousarray(
                np.asarray(Wv_b, np.float32)[idx].reshape(1, JC)),
            "ONESR": np.ones((1, 128), np.float32),
        })
    return in_maps


_prog_cache = {}


def kernel(X, Wq_w, Wq_b, Wk_w, Wk_b, Wv_w, Wv_b, Wo_w, Wo_b, _trace=False):
    from concourse.bass_utils import run_bass_kernel_spmd

    if "nc" not in _prog_cache:
        _prog_cache["nc"] = build_program()
    nc = _prog_cache["nc"]
    in_maps = make_core_inputs(X, Wq_w, Wq_b, Wk_w, Wk_b, Wv_w, Wv_b, Wo_w)
    res = run_bass_kernel_spmd(nc, in_maps, core_ids=list(range(NCORES)),
                               trace=_trace)
    y = np.zeros((L, D), np.float64)
    for r in res.results:
        y += r["Y"].astype(np.float64)
    y += np.asarray(Wo_b, np.float32).astype(np.float64)
    out = y.astype(np.float32)
    if _trace:
        kernel.last_results = res
    return out


# revision 16
# speedup vs baseline: 1.2129x; 1.2129x over previous
"""Trainium2 Bass kernel for nn_MultiHeadAttention_62319975465542.

Tensor-parallel over heads (Megatron-style): 32 heads sharded 4-per-core
across 8 NeuronCores. Each core computes its heads' QKV projections,
attention, and a partial output projection; the host sums the 8 partials
(the all-reduce after Wo) and adds the output bias.

Reference layout note: Q = (X @ Wq.T + b).reshape(L, D_HEAD, NUM_HEADS),
so head h owns interleaved feature columns {d*32 + h : d in 0..63}. The
host pre-gathers those columns into contiguous per-core blocks.

All matmuls run as float32r (TF32-like, 1 cycle/row for N>=256); every
tensor feeding a matmul is declared float32r end-to-end (the compiler
requires producers to round to f32r). The numpy byte layout is float32.
"""

import numpy as np

import concourse.bass as bass
import concourse.tile as tile
import concourse.mybir as mybir
from concourse import bacc

F32 = mybir.dt.float32
F32R = mybir.dt.float32r
BF16 = mybir.dt.bfloat16
Identity = mybir.ActivationFunctionType.Identity
Exp = mybir.ActivationFunctionType.Exp

L = 2048          # sequence length
D = 2048          # d_model
NH = 32           # total heads
DH = 64           # head dim
NCORES = 8
HPC = NH // NCORES   # heads per core = 4
JC = HPC * DH        # per-core projected width = 256
LB = 512             # l-block width
NLB = L // LB        # 4
KO = D // 128        # 16 contraction chunks
MC = L // 128        # 16 key chunks


def build_program():
    nc = bacc.Bacc("TRN2", target_bir_lowering=False, debug=False)

    xt_d = nc.dram_tensor("XT", (D, L), F32R, kind="ExternalInput")
    wq_d = nc.dram_tensor("WQ", (128, KO, JC), F32R, kind="ExternalInput")
    wk_d = nc.dram_tensor("WK", (128, KO, JC), F32R, kind="ExternalInput")
    wv_d = nc.dram_tensor("WV", (128, KO, JC), F32R, kind="ExternalInput")
    wo_d = nc.dram_tensor("WO", (128, 2, D), F32R, kind="ExternalInput")
    bq_d = nc.dram_tensor("BQ", (128, 2), F32, kind="ExternalInput")
    bk_d = nc.dram_tensor("BK", (128, 2), F32, kind="ExternalInput")
    bv_d = nc.dram_tensor("BV", (1, JC), F32R, kind="ExternalInput")
    onesr_d = nc.dram_tensor("ONESR", (1, 128), F32R, kind="ExternalInput")
    y_d = nc.dram_tensor("Y", (L, D), F32, kind="ExternalOutput")

    with tile.TileContext(nc) as tc, nc.allow_low_precision(
            reason="float32r outputs are fp32-width; rounding is intended"):
        with tc.tile_pool(name="const", bufs=1) as cp:
            wq_sb = cp.tile((128, KO, JC), F32R)
            wk_sb = cp.tile((128, KO, JC), F32R)
            wv_sb = cp.tile((128, KO, JC), F32R)
            wo_sb = cp.tile((128, 2, D), F32R)
            bq_sb = cp.tile((128, 2), F32)
            bk_sb = cp.tile((128, 2), F32)
            bv_sb = cp.tile((1, JC), F32R)
            ones_sb = cp.tile((128, 128), BF16)
            onesr_sb = cp.tile((1, 128), F32R)
            nc.sync.dma_start(wq_sb[:], wq_d[:])
            nc.sync.dma_start(wk_sb[:], wk_d[:])
            nc.sync.dma_start(wv_sb[:], wv_d[:])
            nc.sync.dma_start(wo_sb[:], wo_d[:])
            nc.sync.dma_start(bq_sb[:], bq_d[:])
            nc.sync.dma_start(bk_sb[:], bk_d[:])
            nc.sync.dma_start(bv_sb[:], bv_d[:])
            nc.sync.dma_start(onesr_sb[:], onesr_d[:])
            nc.vector.memset(ones_sb[:], 1.0)

            # persistent activations
            qt_sb = [cp.tile((128, L), F32R, name=f"qt{p}") for p in range(2)]
            kt_sb = [cp.tile((128, L), F32R, name=f"kt{p}") for p in range(2)]
            v_sb = cp.tile((128, MC, HPC * 65), BF16)
            nc.vector.memset(v_sb[:], 1.0)
            ot_sb = [cp.tile((128, L), F32R, name=f"ot{p}") for p in range(2)]

            # ---------------- Phase 1: QKV projections ----------------
            with (
                tc.tile_pool(name="xt", bufs=2) as xtp,
                tc.tile_pool(name="qkps", bufs=4, space="PSUM") as qkps,
                tc.tile_pool(name="vps", bufs=4, space="PSUM") as vps,
            ):
                for lb in range(NLB):
                    xt = xtp.tile((128, KO, LB), F32R, name="xt_t")
                    for ko in range(KO):
                        nc.sync.dma_start(
                            xt[:, ko, :],
                            xt_d[ko * 128:(ko + 1) * 128, lb * LB:(lb + 1) * LB],
                        )
                    # K^T then Q^T (both (j, l) layout, bias per partition j)
                    for w_sb, b_sb, dst in ((wk_sb, bk_sb, kt_sb),
                                            (wq_sb, bq_sb, qt_sb)):
                        for jc in range(2):
                            ps = qkps.tile((128, LB), F32, name="qk_ps")
                            for ko in range(KO):
                                nc.tensor.matmul(
                                    ps[:],
                                    (w_sb[:, ko, jc * 128:(jc + 1) * 128]),
                                    (xt[:, ko, :]),
                                    start=(ko == 0), stop=(ko == KO - 1),
                                )
                            nc.scalar.activation(
                                dst[jc][:, lb * LB:(lb + 1) * LB], ps[:],
                                Identity, bias=b_sb[:, jc:jc + 1],
                            )
                    # V in (l, j) layout; bias via K=1 ones-row matmul
                    for lt in range(4):
                        vp = vps.tile((128, JC), F32, name="v_ps")
                        for ko in range(KO):
                            nc.tensor.matmul(
                                vp[:],
                                (xt[:, ko, lt * 128:(lt + 1) * 128]),
                                (wv_sb[:, ko, :]),
                                start=(ko == 0), stop=False,
                            )
                        nc.tensor.matmul(
                            vp[:], (onesr_sb[0:1, :]), (bv_sb[0:1, :]),
                            start=False, stop=True,
                        )
                        nc.vector.tensor_copy(
                            v_sb[:, lb * 4 + lt, :].rearrange(
                                "p (h e) -> p h e", h=HPC)[:, :, 0:DH],
                            vp[:].rearrange("p (h d) -> p h d", h=HPC))

            # ---------------- Phase 2: attention + out-projection ----------------
            with (
                tc.tile_pool(name="epool", bufs=3) as epool,
                tc.tile_pool(name="norm", bufs=2) as normp,
                tc.tile_pool(name="ysb", bufs=3) as ypool,
                tc.tile_pool(name="scps", bufs=2, space="PSUM") as scps,
                tc.tile_pool(name="accps", bufs=4, space="PSUM") as accps,
            ):
                for lb in range(NLB):
                    lsl = slice(lb * LB, (lb + 1) * LB)
                    for p in range(2):  # head pair (local heads 2p, 2p+1)
                        # per-head accumulators; [V_h | 1] lhsT gives rows
                        # 0:64 = unnormalized A·V, row 64 = softmax denominator
                        ava = accps.tile((128, LB), F32, name="acc_ps")
                        avb = accps.tile((128, LB), F32, name="acc_ps")
                        for m in range(MC):
                            msl = slice(m * 128, (m + 1) * 128)
                            sc = scps.tile((128, 2 * LB), F32, name="sc_ps")
                            # S^T = K·Q^T per head; heads at partition 0:64/64:128
                            nc.tensor.matmul(
                                sc[:, 0:LB],
                                (kt_sb[p][0:64, msl]), (qt_sb[p][0:64, lsl]),
                            )
                            nc.tensor.matmul(
                                sc[:, LB:2 * LB],
                                (kt_sb[p][64:128, msl]), (qt_sb[p][64:128, lsl]),
                            )
                            e = epool.tile((128, 2 * LB), BF16, name="e_sb")
                            nc.scalar.activation(e[:], sc[:], Exp)
                            # A·V per head with [V_h | 1] lhsT (M=65):
                            # rows 0:64 = unnormalized output, row 64 = denom
                            nc.tensor.matmul(
                                ava[0:65, :],
                                (v_sb[:, m, (2 * p) * 65:(2 * p) * 65 + 65]),
                                (e[:, 0:LB]),
                                start=(m == 0), stop=(m == MC - 1),
                            )
                            nc.tensor.matmul(
                                avb[0:65, :],
                                (v_sb[:, m, (2 * p + 1) * 65:(2 * p + 1) * 65 + 65]),
                                (e[:, LB:2 * LB]),
                                start=(m == 0), stop=(m == MC - 1),
                            )
                        # normalize: recip of denominator rows, DMA partition-
                        # broadcast to 64 rows, then elementwise multiply
                        ra = normp.tile((1, LB), F32, name="ra_sb")
                        rb = normp.tile((1, LB), F32, name="rb_sb")
                        nc.vector.reciprocal(ra[:], ava[64:65, :])
                        nc.vector.reciprocal(rb[:], avb[64:65, :])
                        rba = normp.tile((64, LB), F32, name="rba_sb")
                        rbb = normp.tile((64, LB), F32, name="rbb_sb")
                        nc.gpsimd.partition_broadcast(rba[:], ra[:])
                        nc.gpsimd.partition_broadcast(rbb[:], rb[:])
                        nc.vector.tensor_tensor(
                            ot_sb[p][0:64, lsl], ava[0:64, :], rba[:],
                            mybir.AluOpType.mult,
                        )
                        nc.vector.tensor_tensor(
                            ot_sb[p][64:128, lsl], avb[0:64, :], rbb[:],
                            mybir.AluOpType.mult,
                        )
                    # out-projection for this l-block
                    for lt in range(4):
                        row0 = lb * LB + lt * 128
                        for ns in range(4):
                            yp = accps.tile((128, 512), F32, name="acc_ps")
                            for jc in range(2):
                                nc.tensor.matmul(
                                    yp[:],
                                    (ot_sb[jc][:, row0:row0 + 128]),
                                    (wo_sb[:, jc, ns * 512:(ns + 1) * 512]),
                                    start=(jc == 0), stop=(jc == 1),
                                )
                            ty = ypool.tile((128, 512), F32, name="y_sb")
                            nc.any.tensor_copy(ty[:], yp[:])
                            nc.sync.dma_start(
                                y_d[row0:row0 + 128, ns * 512:(ns + 1) * 512],
                                ty[:],
                            )

    nc.compile()
    return nc


def make_core_inputs(X, Wq_w, Wq_b, Wk_w, Wk_b, Wv_w, Wv_b, Wo_w):
    """Host-side sharding: per-core input dicts (shared XT + per-core weights)."""
    X = np.asarray(X, np.float32)
    xt = np.ascontiguousarray(X.T)
    scale = 1.0 / np.sqrt(np.float32(D))
    in_maps = []
    for c in range(NCORES):
        idx = np.array([d * NH + h for h in range(c * HPC, (c + 1) * HPC)
                        for d in range(DH)], np.int64)

        def kxj(w, s=1.0):
            # (D_in=K, JC) -> (128, KO, JC) with [p, ko, j] = w.T[ko*128+p, j]
            wt = np.ascontiguousarray((np.asarray(w, np.float32)[idx, :] * s).T)
            return np.ascontiguousarray(wt.reshape(KO, 128, JC).transpose(1, 0, 2))

        wo = np.ascontiguousarray(np.asarray(Wo_w, np.float32)[:, idx].T)  # (JC, D)
        wo = np.ascontiguousarray(wo.reshape(2, 128, D).transpose(1, 0, 2))

        def bcol(b, s=1.0):
            return np.ascontiguousarray(
                (np.asarray(b, np.float32)[idx] * s).reshape(2, 128).T)

        in_maps.append({
            "XT": xt,
            "WQ": kxj(Wq_w, scale), "WK": kxj(Wk_w), "WV": kxj(Wv_w),
            "WO": wo,
            "BQ": bcol(Wq_b, scale), "BK": bcol(Wk_b),
            "BV": np.ascontiguousarray(
                np.asarray(Wv_b, np.float32)[idx].reshape(1, JC)),
            "ONESR": np.ones((1, 128), np.float32),
        })
    return in_maps


_prog_cache = {}


def kernel(X, Wq_w, Wq_b, Wk_w, Wk_b, Wv_w, Wv_b, Wo_w, Wo_b, _trace=False):
    from concourse.bass_utils import run_bass_kernel_spmd

    if "nc" not in _prog_cache:
        _prog_cache["nc"] = build_program()
    nc = _prog_cache["nc"]
    in_maps = make_core_inputs(X, Wq_w, Wq_b, Wk_w, Wk_b, Wv_w, Wv_b, Wo_w)
    res = run_bass_kernel_spmd(nc, in_maps, core_ids=list(range(NCORES)),
                               trace=_trace)
    y = np.zeros((L, D), np.float64)
    for r in res.results:
        y += r["Y"].astype(np.float64)
    y += np.asarray(Wo_b, np.float32).astype(np.float64)
    out = y.astype(np.float32)
    if _trace:
        kernel.last_results = res
    return out


# revision 17
# speedup vs baseline: 1.2872x; 1.0613x over previous
"""Trainium2 Bass kernel for nn_MultiHeadAttention_62319975465542.

Tensor-parallel over heads (Megatron-style): 32 heads sharded 4-per-core
across 8 NeuronCores. Each core computes its heads' QKV projections,
attention, and a partial output projection; the host sums the 8 partials
(the all-reduce after Wo) and adds the output bias.

Reference layout note: Q = (X @ Wq.T + b).reshape(L, D_HEAD, NUM_HEADS),
so head h owns interleaved feature columns {d*32 + h : d in 0..63}. The
host pre-gathers those columns into contiguous per-core blocks.

v3: all matmul inputs in bf16 (fast weight load, halved DMA), X^T fully
SBUF-resident, and emission ordered so ScalarE exp for one l-block
overlaps TensorE work of the next (Q(lb0) first, then K/V, then
attention blocks). Softmax denominators ride in the A-V matmul as a
17th "ones" V-column ([V_h | 1], M=65); normalization is reciprocal +
gpsimd partition-broadcast + one vector multiply. PSUM stays fp32.
"""

import numpy as np
import ml_dtypes

import concourse.bass as bass
import concourse.tile as tile
import concourse.mybir as mybir
from concourse import bacc

F32 = mybir.dt.float32
BF16 = mybir.dt.bfloat16
Identity = mybir.ActivationFunctionType.Identity
Exp = mybir.ActivationFunctionType.Exp
MULT = mybir.AluOpType.mult
ADD = mybir.AluOpType.add

L = 2048          # sequence length
D = 2048          # d_model
NH = 32           # total heads
DH = 64           # head dim
NCORES = 8
HPC = NH // NCORES   # heads per core = 4
JC = HPC * DH        # per-core projected width = 256
LB = 512             # l-block width
NLB = L // LB        # 4
KO = D // 128        # 16 contraction chunks
MC = L // 128        # 16 key chunks


def build_program():
    nc = bacc.Bacc("TRN2", target_bir_lowering=False, debug=False)

    xt_d = nc.dram_tensor("XT", (D, L), BF16, kind="ExternalInput")
    wq_d = nc.dram_tensor("WQ", (128, KO, JC), BF16, kind="ExternalInput")
    wk_d = nc.dram_tensor("WK", (128, KO, JC), BF16, kind="ExternalInput")
    wv_d = nc.dram_tensor("WV", (128, KO, JC), BF16, kind="ExternalInput")
    wo_d = nc.dram_tensor("WO", (128, 2, D), BF16, kind="ExternalInput")
    bq_d = nc.dram_tensor("BQ", (128, 2), F32, kind="ExternalInput")
    bk_d = nc.dram_tensor("BK", (128, 2), F32, kind="ExternalInput")
    bv_d = nc.dram_tensor("BV", (1, JC), BF16, kind="ExternalInput")
    ones_d = nc.dram_tensor("ONES", (1, 128), BF16, kind="ExternalInput")
    y_d = nc.dram_tensor("Y", (L, D), F32, kind="ExternalOutput")

    with tile.TileContext(nc) as tc, nc.allow_low_precision(
            reason="bf16 activations are within tolerance for this op"):
        with (
            tc.tile_pool(name="const", bufs=1) as cp,
            tc.tile_pool(name="epool", bufs=3) as epool,
            tc.tile_pool(name="norm", bufs=2) as normp,
            tc.tile_pool(name="ysb", bufs=3) as ypool,
            tc.tile_pool(name="scps", bufs=2, space="PSUM") as scps,
            tc.tile_pool(name="accps", bufs=4, space="PSUM") as accps,
        ):
            wq_sb = cp.tile((128, KO, JC), BF16)
            wk_sb = cp.tile((128, KO, JC), BF16)
            wv_sb = cp.tile((128, KO, JC), BF16)
            wo_sb = cp.tile((128, 2, D), BF16)
            bq_sb = cp.tile((128, 2), F32)
            bk_sb = cp.tile((128, 2), F32)
            bv_sb = cp.tile((1, JC), BF16)
            ones_sb = cp.tile((1, 128), BF16)
            xt_sb = cp.tile((128, KO, L), BF16)
            nc.sync.dma_start(wq_sb[:], wq_d[:])
            nc.sync.dma_start(wk_sb[:], wk_d[:])
            nc.sync.dma_start(wv_sb[:], wv_d[:])
            nc.sync.dma_start(wo_sb[:], wo_d[:])
            nc.sync.dma_start(bq_sb[:], bq_d[:])
            nc.sync.dma_start(bk_sb[:], bk_d[:])
            nc.sync.dma_start(bv_sb[:], bv_d[:])
            nc.sync.dma_start(ones_sb[:], ones_d[:])

            qt_sb = [cp.tile((128, L), BF16, name=f"qt{p}") for p in range(2)]
            kt_sb = [cp.tile((128, L), BF16, name=f"kt{p}") for p in range(2)]
            v_sb = cp.tile((128, MC, HPC * 65), BF16)
            nc.vector.memset(v_sb[:], 1.0)
            ot_sb = [cp.tile((128, L), BF16, name=f"ot{p}") for p in range(2)]

            # X^T loads: per (ko, lb) slice so consumers can start early
            for ko in range(KO):
                for lb in range(NLB):
                    nc.sync.dma_start(
                        xt_sb[:, ko, lb * LB:(lb + 1) * LB],
                        xt_d[ko * 128:(ko + 1) * 128, lb * LB:(lb + 1) * LB],
                    )

            def proj_qk(w_sb, b_sb, dst, lb):
                """(j, l) layout projection with per-partition bias via DVE."""
                for jc in range(2):
                    ps = accps.tile((128, LB), F32, name="acc_ps")
                    for ko in range(KO):
                        nc.tensor.matmul(
                            ps[:],
                            w_sb[:, ko, jc * 128:(jc + 1) * 128],
                            xt_sb[:, ko, lb * LB:(lb + 1) * LB],
                            start=(ko == 0), stop=(ko == KO - 1),
                        )
                    nc.vector.tensor_scalar(
                        dst[jc][:, lb * LB:(lb + 1) * LB], ps[:],
                        b_sb[:, jc:jc + 1], None, ADD,
                    )

            def proj_v(lb):
                """V in (l, j) layout; bias via K=1 ones-row matmul."""
                for lt in range(4):
                    vp = accps.tile((128, LB), F32, name="acc_ps")
                    for ko in range(KO):
                        nc.tensor.matmul(
                            vp[:, 0:JC],
                            xt_sb[:, ko, lb * LB + lt * 128:lb * LB + (lt + 1) * 128],
                            wv_sb[:, ko, :],
                            start=(ko == 0), stop=False,
                        )
                    nc.tensor.matmul(
                        vp[:, 0:JC], ones_sb[0:1, :], bv_sb[0:1, :],
                        start=False, stop=True,
                    )
                    nc.vector.tensor_copy(
                        v_sb[:, lb * 4 + lt, :].rearrange(
                            "p (h e) -> p h e", h=HPC)[:, :, 0:DH],
                        vp[:, 0:JC].rearrange("p (h d) -> p h d", h=HPC))

            def attention(lb):
                lsl = slice(lb * LB, (lb + 1) * LB)
                for p in range(2):  # head pair (local heads 2p, 2p+1)
                    ava = accps.tile((128, LB), F32, name="acc_ps")
                    avb = accps.tile((128, LB), F32, name="acc_ps")
                    for m in range(MC):
                        msl = slice(m * 128, (m + 1) * 128)
                        sc = scps.tile((128, 2 * LB), F32, name="sc_ps")
                        nc.tensor.matmul(
                            sc[:, 0:LB],
                            kt_sb[p][0:64, msl], qt_sb[p][0:64, lsl],
                        )
                        nc.tensor.matmul(
                            sc[:, LB:2 * LB],
                            kt_sb[p][64:128, msl], qt_sb[p][64:128, lsl],
                        )
                        e = epool.tile((128, 2 * LB), BF16, name="e_sb")
                        nc.scalar.activation(e[:], sc[:], Exp)
                        # A·V per head, [V_h | 1] lhsT: row 64 = denominator
                        nc.tensor.matmul(
                            ava[0:65, :],
                            v_sb[:, m, (2 * p) * 65:(2 * p) * 65 + 65],
                            e[:, 0:LB],
                            start=(m == 0), stop=(m == MC - 1),
                        )
                        nc.tensor.matmul(
                            avb[0:65, :],
                            v_sb[:, m, (2 * p + 1) * 65:(2 * p + 1) * 65 + 65],
                            e[:, LB:2 * LB],
                            start=(m == 0), stop=(m == MC - 1),
                        )
                    ra = normp.tile((1, LB), F32, name="ra_sb")
                    rb = normp.tile((1, LB), F32, name="rb_sb")
                    nc.vector.reciprocal(ra[:], ava[64:65, :])
                    nc.vector.reciprocal(rb[:], avb[64:65, :])
                    rba = normp.tile((64, LB), F32, name="rba_sb")
                    rbb = normp.tile((64, LB), F32, name="rbb_sb")
                    nc.gpsimd.partition_broadcast(rba[:], ra[:])
                    nc.gpsimd.partition_broadcast(rbb[:], rb[:])
                    nc.vector.tensor_tensor(
                        ot_sb[p][0:64, lsl], ava[0:64, :], rba[:], MULT)
                    nc.vector.tensor_tensor(
                        ot_sb[p][64:128, lsl], avb[0:64, :], rbb[:], MULT)

            def outproj(lb):
                for lt in range(4):
                    row0 = lb * LB + lt * 128
                    for ns in range(4):
                        yp = accps.tile((128, 512), F32, name="acc_ps")
                        for jc in range(2):
                            nc.tensor.matmul(
                                yp[:],
                                ot_sb[jc][:, row0:row0 + 128],
                                wo_sb[:, jc, ns * 512:(ns + 1) * 512],
                                start=(jc == 0), stop=(jc == 1),
                            )
                        ty = ypool.tile((128, 512), F32, name="y_sb")
                        nc.any.tensor_copy(ty[:], yp[:])
                        nc.sync.dma_start(
                            y_d[row0:row0 + 128, ns * 512:(ns + 1) * 512],
                            ty[:],
                        )

            # Emission order: Q(lb0) early so attention lb0's exps can
            # start as soon as K/V chunks land; K/V per l-block; then the
            # per-lb stream Q(lb+1) -> attention(lb) -> outproj(lb).
            proj_qk(wq_sb, bq_sb, qt_sb, 0)
            for lb in range(NLB):
                proj_qk(wk_sb, bk_sb, kt_sb, lb)
                proj_v(lb)
            for lb in range(NLB):
                if lb + 1 < NLB:
                    proj_qk(wq_sb, bq_sb, qt_sb, lb + 1)
                attention(lb)
                outproj(lb)

    nc.compile()
    return nc


def make_core_inputs(X, Wq_w, Wq_b, Wk_w, Wk_b, Wv_w, Wv_b, Wo_w):
    """Host-side sharding: per-core input dicts (shared XT + per-core weights)."""
    X = np.asarray(X, np.float32)
    bf = ml_dtypes.bfloat16
    xt = np.ascontiguousarray(X.T).astype(bf)
    scale = 1.0 / np.sqrt(np.float32(D))
    in_maps = []
    for c in range(NCORES):
        idx = np.array([d * NH + h for h in range(c * HPC, (c + 1) * HPC)
                        for d in range(DH)], np.int64)

        def kxj(w, s=1.0):
            # (D_in=K, JC) -> (128, KO, JC) with [p, ko, j] = w.T[ko*128+p, j]
            wt = np.ascontiguousarray((np.asarray(w, np.float32)[idx, :] * s).T)
            return np.ascontiguousarray(
                wt.reshape(KO, 128, JC).transpose(1, 0, 2)).astype(bf)

        wo = np.ascontiguousarray(np.asarray(Wo_w, np.float32)[:, idx].T)  # (JC, D)
        wo = np.ascontiguousarray(wo.reshape(2, 128, D).transpose(1, 0, 2)).astype(bf)

        def bcol(b, s=1.0):
            return np.ascontiguousarray(
                (np.asarray(b, np.float32)[idx] * s).reshape(2, 128).T)

        in_maps.append({
            "XT": xt,
            "WQ": kxj(Wq_w, scale), "WK": kxj(Wk_w), "WV": kxj(Wv_w),
            "WO": wo,
            "BQ": bcol(Wq_b, scale), "BK": bcol(Wk_b),
            "BV": np.asarray(Wv_b, np.float32)[idx].reshape(1, JC).astype(bf),
            "ONES": np.ones((1, 128), bf),
        })
    return in_maps


_prog_cache = {}


def kernel(X, Wq_w, Wq_b, Wk_w, Wk_b, Wv_w, Wv_b, Wo_w, Wo_b, _trace=False):
    from concourse.bass_utils import run_bass_kernel_spmd

    if "nc" not in _prog_cache:
        _prog_cache["nc"] = build_program()
    nc = _prog_cache["nc"]
    in_maps = make_core_inputs(X, Wq_w, Wq_b, Wk_w, Wk_b, Wv_w, Wv_b, Wo_w)
    res = run_bass_kernel_spmd(nc, in_maps, core_ids=list(range(NCORES)),
                               trace=_trace)
    y = np.zeros((L, D), np.float64)
    for r in res.results:
        y += r["Y"].astype(np.float64)
    y += np.asarray(Wo_b, np.float32).astype(np.float64)
    out = y.astype(np.float32)
    if _trace:
        kernel.last_results = res
    return out


# revision 18
# speedup vs baseline: 1.4463x; 1.1236x over previous
"""Trainium2 Bass kernel for nn_MultiHeadAttention_62319975465542.

Tensor-parallel over heads (Megatron-style): 32 heads sharded 4-per-core
across 8 NeuronCores. Each core computes its heads' QKV projections,
attention, and a partial output projection; the host sums the 8 partials
(the all-reduce after Wo) and adds the output bias.

Reference layout note: Q = (X @ Wq.T + b).reshape(L, D_HEAD, NUM_HEADS),
so head h owns interleaved feature columns {d*32 + h : d in 0..63}. The
host pre-gathers those columns into contiguous per-core blocks.

v3: all matmul inputs in bf16 (fast weight load, halved DMA), X^T fully
SBUF-resident, and emission ordered so ScalarE exp for one l-block
overlaps TensorE work of the next (Q(lb0) first, then K/V, then
attention blocks). Softmax denominators ride in the A-V matmul as a
17th "ones" V-column ([V_h | 1], M=65); normalization is reciprocal +
gpsimd partition-broadcast + one vector multiply. PSUM stays fp32.
"""

import numpy as np
import ml_dtypes

import concourse.bass as bass
import concourse.tile as tile
import concourse.mybir as mybir
from concourse import bacc

F32 = mybir.dt.float32
BF16 = mybir.dt.bfloat16
Identity = mybir.ActivationFunctionType.Identity
Exp = mybir.ActivationFunctionType.Exp
MULT = mybir.AluOpType.mult
ADD = mybir.AluOpType.add

L = 2048          # sequence length
D = 2048          # d_model
NH = 32           # total heads
DH = 64           # head dim
NCORES = 8
HPC = NH // NCORES   # heads per core = 4
JC = HPC * DH        # per-core projected width = 256
LB = 512             # l-block width
NLB = L // LB        # 4
KO = D // 128        # 16 contraction chunks
MC = L // 128        # 16 key chunks


def build_program():
    nc = bacc.Bacc("TRN2", target_bir_lowering=False, debug=False)

    xt_d = nc.dram_tensor("XT", (D, L), BF16, kind="ExternalInput")
    wq_d = nc.dram_tensor("WQ", (128, KO, JC), BF16, kind="ExternalInput")
    wk_d = nc.dram_tensor("WK", (128, KO, JC), BF16, kind="ExternalInput")
    wv_d = nc.dram_tensor("WV", (128, KO, JC), BF16, kind="ExternalInput")
    wo_d = nc.dram_tensor("WO", (128, 2, D), BF16, kind="ExternalInput")
    bq_d = nc.dram_tensor("BQ", (128, 2), F32, kind="ExternalInput")
    bk_d = nc.dram_tensor("BK", (128, 2), F32, kind="ExternalInput")
    bv_d = nc.dram_tensor("BV", (1, JC), BF16, kind="ExternalInput")
    ones_d = nc.dram_tensor("ONES", (1, 128), BF16, kind="ExternalInput")
    y_d = nc.dram_tensor("Y", (L, D), F32, kind="ExternalOutput")

    with tile.TileContext(nc) as tc, nc.allow_low_precision(
            reason="bf16 activations are within tolerance for this op"):
        with (
            tc.tile_pool(name="const", bufs=1) as cp,
            tc.tile_pool(name="epool", bufs=3) as epool,
            tc.tile_pool(name="norm", bufs=2) as normp,
            tc.tile_pool(name="ysb", bufs=3) as ypool,
            tc.tile_pool(name="scps", bufs=2, space="PSUM") as scps,
            tc.tile_pool(name="accps", bufs=4, space="PSUM") as accps,
        ):
            wq_sb = cp.tile((128, KO, JC), BF16)
            wk_sb = cp.tile((128, KO, JC), BF16)
            wv_sb = cp.tile((128, KO, JC), BF16)
            wo_sb = cp.tile((128, 2, D), BF16)
            bq_sb = cp.tile((128, 2), F32)
            bk_sb = cp.tile((128, 2), F32)
            bv_sb = cp.tile((1, JC), BF16)
            ones_sb = cp.tile((1, 128), BF16)
            xt_sb = cp.tile((128, KO, L), BF16)
            nc.sync.dma_start(wq_sb[:], wq_d[:])
            nc.sync.dma_start(wk_sb[:], wk_d[:])
            nc.sync.dma_start(wv_sb[:], wv_d[:])
            nc.sync.dma_start(wo_sb[:], wo_d[:])
            nc.sync.dma_start(bq_sb[:], bq_d[:])
            nc.sync.dma_start(bk_sb[:], bk_d[:])
            nc.sync.dma_start(bv_sb[:], bv_d[:])
            nc.sync.dma_start(ones_sb[:], ones_d[:])

            qt_sb = [cp.tile((128, L), BF16, name=f"qt{p}") for p in range(2)]
            kt_sb = [cp.tile((128, L), BF16, name=f"kt{p}") for p in range(2)]
            v_sb = cp.tile((128, MC, HPC * 65), BF16)
            nc.vector.memset(v_sb[:], 1.0)
            ot_sb = [cp.tile((128, L), BF16, name=f"ot{p}") for p in range(2)]

            # X^T loads: per (ko, lb) slice so consumers can start early
            for ko in range(KO):
                for lb in range(NLB):
                    nc.sync.dma_start(
                        xt_sb[:, ko, lb * LB:(lb + 1) * LB],
                        xt_d[ko * 128:(ko + 1) * 128, lb * LB:(lb + 1) * LB],
                    )

            def proj_qk(w_sb, b_sb, dst, lb):
                """(j, l) layout projection with per-partition bias via DVE."""
                for jc in range(2):
                    ps = accps.tile((128, LB), F32, name="acc_ps")
                    for ko in range(KO):
                        nc.tensor.matmul(
                            ps[:],
                            w_sb[:, ko, jc * 128:(jc + 1) * 128],
                            xt_sb[:, ko, lb * LB:(lb + 1) * LB],
                            start=(ko == 0), stop=(ko == KO - 1),
                        )
                    nc.vector.tensor_scalar(
                        dst[jc][:, lb * LB:(lb + 1) * LB], ps[:],
                        b_sb[:, jc:jc + 1], None, ADD,
                    )

            def proj_v(lb):
                """V in (l, j) layout; bias via K=1 ones-row matmul."""
                for lt in range(4):
                    vp = accps.tile((128, LB), F32, name="acc_ps")
                    for ko in range(KO):
                        nc.tensor.matmul(
                            vp[:, 0:JC],
                            xt_sb[:, ko, lb * LB + lt * 128:lb * LB + (lt + 1) * 128],
                            wv_sb[:, ko, :],
                            start=(ko == 0), stop=False,
                        )
                    nc.tensor.matmul(
                        vp[:, 0:JC], ones_sb[0:1, :], bv_sb[0:1, :],
                        start=False, stop=True,
                    )
                    nc.vector.tensor_copy(
                        v_sb[:, lb * 4 + lt, :].rearrange(
                            "p (h e) -> p h e", h=HPC)[:, :, 0:DH],
                        vp[:, 0:JC].rearrange("p (h d) -> p h d", h=HPC))

            def attention(lb, filler):
                """Attention for one l-block; pops one closure from
                `filler` per m-chunk to keep PE dense while ScalarE runs
                the exps (exp is slower per chunk than the paired MMs)."""
                lsl = slice(lb * LB, (lb + 1) * LB)
                for p in range(2):  # head pair (local heads 2p, 2p+1)
                    ava = accps.tile((128, LB), F32, name="acc_ps")
                    avb = accps.tile((128, LB), F32, name="acc_ps")
                    for m in range(MC):
                        msl = slice(m * 128, (m + 1) * 128)
                        sc = scps.tile((128, 2 * LB), F32, name="sc_ps")
                        nc.tensor.matmul(
                            sc[:, 0:LB],
                            kt_sb[p][0:64, msl], qt_sb[p][0:64, lsl],
                        )
                        nc.tensor.matmul(
                            sc[:, LB:2 * LB],
                            kt_sb[p][64:128, msl], qt_sb[p][64:128, lsl],
                        )
                        e = epool.tile((128, 2 * LB), BF16, name="e_sb")
                        nc.scalar.activation(e[:], sc[:], Exp)
                        # A·V per head, [V_h | 1] lhsT: row 64 = denominator
                        nc.tensor.matmul(
                            ava[0:65, :],
                            v_sb[:, m, (2 * p) * 65:(2 * p) * 65 + 65],
                            e[:, 0:LB],
                            start=(m == 0), stop=(m == MC - 1),
                        )
                        nc.tensor.matmul(
                            avb[0:65, :],
                            v_sb[:, m, (2 * p + 1) * 65:(2 * p + 1) * 65 + 65],
                            e[:, LB:2 * LB],
                            start=(m == 0), stop=(m == MC - 1),
                        )
                        if filler:
                            filler.pop(0)()
                    # copy raw A·V (+denominator row) to SBUF, freeing the
                    # PSUM accumulators immediately; normalize from SBUF
                    sva = normp.tile((65, LB), F32, name="sva_sb")
                    svb = normp.tile((65, LB), F32, name="svb_sb")
                    nc.vector.tensor_copy(sva[:], ava[0:65, :])
                    nc.vector.tensor_copy(svb[:], avb[0:65, :])
                    ra = normp.tile((1, LB), F32, name="ra_sb")
                    rb = normp.tile((1, LB), F32, name="rb_sb")
                    nc.vector.reciprocal(ra[:], sva[64:65, :])
                    nc.vector.reciprocal(rb[:], svb[64:65, :])
                    rba = normp.tile((64, LB), F32, name="rba_sb")
                    rbb = normp.tile((64, LB), F32, name="rbb_sb")
                    nc.gpsimd.partition_broadcast(rba[:], ra[:])
                    nc.gpsimd.partition_broadcast(rbb[:], rb[:])
                    nc.vector.tensor_tensor(
                        ot_sb[p][0:64, lsl], sva[0:64, :], rba[:], MULT)
                    nc.vector.tensor_tensor(
                        ot_sb[p][64:128, lsl], svb[0:64, :], rbb[:], MULT)

            def outproj_tile(lb, lt, ns):
                row0 = lb * LB + lt * 128
                yp = accps.tile((128, 512), F32, name="acc_ps")
                for jc in range(2):
                    nc.tensor.matmul(
                        yp[:],
                        ot_sb[jc][:, row0:row0 + 128],
                        wo_sb[:, jc, ns * 512:(ns + 1) * 512],
                        start=(jc == 0), stop=(jc == 1),
                    )
                ty = ypool.tile((128, 512), F32, name="y_sb")
                nc.any.tensor_copy(ty[:], yp[:])
                nc.sync.dma_start(
                    y_d[row0:row0 + 128, ns * 512:(ns + 1) * 512], ty[:])

            def qk_group(w_sb, b_sb, dst, lb, jc):
                ps = accps.tile((128, LB), F32, name="acc_ps")
                for ko in range(KO):
                    nc.tensor.matmul(
                        ps[:],
                        w_sb[:, ko, jc * 128:(jc + 1) * 128],
                        xt_sb[:, ko, lb * LB:(lb + 1) * LB],
                        start=(ko == 0), stop=(ko == KO - 1),
                    )
                nc.vector.tensor_scalar(
                    dst[jc][:, lb * LB:(lb + 1) * LB], ps[:],
                    b_sb[:, jc:jc + 1], None, ADD,
                )

            # Emission: Q(lb0), all K/V, then per lb the attention loop
            # with out-projection (lb-1) and Q (lb+1) interleaved as
            # filler inside the m-loops.
            proj_qk(wq_sb, bq_sb, qt_sb, 0)
            for lb in range(NLB):
                proj_qk(wk_sb, bk_sb, kt_sb, lb)
                proj_v(lb)
            for lb in range(NLB):
                filler = []
                if lb > 0:
                    filler += [
                        (lambda lb=lb, lt=lt, ns=ns: outproj_tile(lb - 1, lt, ns))
                        for lt in range(4) for ns in range(4)
                    ]
                if lb + 1 < NLB:
                    filler += [
                        (lambda lb=lb, jc=jc: qk_group(wq_sb, bq_sb, qt_sb,
                                                       lb + 1, jc))
                        for jc in range(2)
                    ]
                attention(lb, filler)
                for f in filler:
                    f()
            for lt in range(4):
                for ns in range(4):
                    outproj_tile(NLB - 1, lt, ns)

    nc.compile()
    return nc


def make_core_inputs(X, Wq_w, Wq_b, Wk_w, Wk_b, Wv_w, Wv_b, Wo_w):
    """Host-side sharding: per-core input dicts (shared XT + per-core weights)."""
    X = np.asarray(X, np.float32)
    bf = ml_dtypes.bfloat16
    xt = np.ascontiguousarray(X.T).astype(bf)
    scale = 1.0 / np.sqrt(np.float32(D))
    in_maps = []
    for c in range(NCORES):
        idx = np.array([d * NH + h for h in range(c * HPC, (c + 1) * HPC)
                        for d in range(DH)], np.int64)

        def kxj(w, s=1.0):
            # (D_in=K, JC) -> (128, KO, JC) with [p, ko, j] = w.T[ko*128+p, j]
            wt = np.ascontiguousarray((np.asarray(w, np.float32)[idx, :] * s).T)
            return np.ascontiguousarray(
                wt.reshape(KO, 128, JC).transpose(1, 0, 2)).astype(bf)

        wo = np.ascontiguousarray(np.asarray(Wo_w, np.float32)[:, idx].T)  # (JC, D)
        wo = np.ascontiguousarray(wo.reshape(2, 128, D).transpose(1, 0, 2)).astype(bf)

        def bcol(b, s=1.0):
            return np.ascontiguousarray(
                (np.asarray(b, np.float32)[idx] * s).reshape(2, 128).T)

        in_maps.append({
            "XT": xt,
            "WQ": kxj(Wq_w, scale), "WK": kxj(Wk_w), "WV": kxj(Wv_w),
            "WO": wo,
            "BQ": bcol(Wq_b, scale), "BK": bcol(Wk_b),
            "BV": np.asarray(Wv_b, np.float32)[idx].reshape(1, JC).astype(bf),
            "ONES": np.ones((1, 128), bf),
        })
    return in_maps


_prog_cache = {}


def kernel(X, Wq_w, Wq_b, Wk_w, Wk_b, Wv_w, Wv_b, Wo_w, Wo_b, _trace=False):
    from concourse.bass_utils import run_bass_kernel_spmd

    if "nc" not in _prog_cache:
        _prog_cache["nc"] = build_program()
    nc = _prog_cache["nc"]
    in_maps = make_core_inputs(X, Wq_w, Wq_b, Wk_w, Wk_b, Wv_w, Wv_b, Wo_w)
    res = run_bass_kernel_spmd(nc, in_maps, core_ids=list(range(NCORES)),
                               trace=_trace)
    y = np.zeros((L, D), np.float64)
    for r in res.results:
        y += r["Y"].astype(np.float64)
    y += np.asarray(Wo_b, np.float32).astype(np.float64)
    out = y.astype(np.float32)
    if _trace:
        kernel.last_results = res
    return out


# revision 21
# speedup vs baseline: 1.5561x; 1.0760x over previous
"""Trainium2 Bass kernel for nn_MultiHeadAttention_62319975465542.

Tensor-parallel over heads (Megatron-style): 32 heads sharded 4-per-core
across 8 NeuronCores. Each core computes its heads' QKV projections,
attention, and a partial output projection; the host sums the 8 partials
(the all-reduce after Wo) and adds the output bias.

Reference layout note: Q = (X @ Wq.T + b).reshape(L, D_HEAD, NUM_HEADS),
so head h owns interleaved feature columns {d*32 + h : d in 0..63}. The
host pre-gathers those columns into contiguous per-core blocks.

v3: all matmul inputs in bf16 (fast weight load, halved DMA), X^T fully
SBUF-resident, and emission ordered so ScalarE exp for one l-block
overlaps TensorE work of the next (Q(lb0) first, then K/V, then
attention blocks). Softmax denominators ride in the A-V matmul as a
17th "ones" V-column ([V_h | 1], M=65); normalization is reciprocal +
gpsimd partition-broadcast + one vector multiply. PSUM stays fp32.
"""

import numpy as np
import ml_dtypes

import concourse.bass as bass
import concourse.tile as tile
import concourse.mybir as mybir
from concourse import bacc

F32 = mybir.dt.float32
BF16 = mybir.dt.bfloat16
Identity = mybir.ActivationFunctionType.Identity
Exp = mybir.ActivationFunctionType.Exp
MULT = mybir.AluOpType.mult
ADD = mybir.AluOpType.add

L = 2048          # sequence length
D = 2048          # d_model
NH = 32           # total heads
DH = 64           # head dim
NCORES = 8
HPC = NH // NCORES   # heads per core = 4
JC = HPC * DH        # per-core projected width = 256
LB = 512             # l-block width
NLB = L // LB        # 4
KO = D // 128        # 16 contraction chunks
MC = L // 128        # 16 key chunks


def build_program():
    nc = bacc.Bacc("TRN2", target_bir_lowering=False, debug=False)

    xt_d = nc.dram_tensor("XT", (D, L), BF16, kind="ExternalInput")
    wq_d = nc.dram_tensor("WQ", (128, KO, JC), BF16, kind="ExternalInput")
    wk_d = nc.dram_tensor("WK", (128, KO, JC), BF16, kind="ExternalInput")
    wv_d = nc.dram_tensor("WV", (128, KO, JC), BF16, kind="ExternalInput")
    wo_d = nc.dram_tensor("WO", (128, 2, D), BF16, kind="ExternalInput")
    bq_d = nc.dram_tensor("BQ", (128, 2), F32, kind="ExternalInput")
    bk_d = nc.dram_tensor("BK", (128, 2), F32, kind="ExternalInput")
    bv_d = nc.dram_tensor("BV", (1, JC), BF16, kind="ExternalInput")
    ones_d = nc.dram_tensor("ONES", (1, 128), BF16, kind="ExternalInput")
    y_d = nc.dram_tensor("Y", (L, D), F32, kind="ExternalOutput")

    with tile.TileContext(nc) as tc, nc.allow_low_precision(
            reason="bf16 activations are within tolerance for this op"):
        with (
            tc.tile_pool(name="const", bufs=1) as cp,
            tc.tile_pool(name="epool", bufs=3) as epool,
            tc.tile_pool(name="norm", bufs=2) as normp,
            tc.tile_pool(name="ysb", bufs=3) as ypool,
            tc.tile_pool(name="scps", bufs=2, space="PSUM") as scps,
            tc.tile_pool(name="accps", bufs=4, space="PSUM") as accps,
        ):
            wq_sb = cp.tile((128, KO, JC), BF16)
            wk_sb = cp.tile((128, KO, JC), BF16)
            wv_sb = cp.tile((128, KO, JC), BF16)
            wo_sb = cp.tile((128, 2, D), BF16)
            bq_sb = cp.tile((128, 2), F32)
            bk_sb = cp.tile((128, 2), F32)
            bv_sb = cp.tile((1, JC), BF16)
            ones_sb = cp.tile((1, 128), BF16)
            xt_sb = cp.tile((128, KO, L), BF16)
            nc.sync.dma_start(wq_sb[:], wq_d[:])
            nc.sync.dma_start(wk_sb[:], wk_d[:])
            nc.sync.dma_start(wv_sb[:], wv_d[:])
            nc.sync.dma_start(wo_sb[:], wo_d[:])
            nc.sync.dma_start(bq_sb[:], bq_d[:])
            nc.sync.dma_start(bk_sb[:], bk_d[:])
            nc.sync.dma_start(bv_sb[:], bv_d[:])
            nc.sync.dma_start(ones_sb[:], ones_d[:])

            qt_sb = [cp.tile((128, L), BF16, name=f"qt{p}") for p in range(2)]
            kt_sb = [cp.tile((128, L), BF16, name=f"kt{p}") for p in range(2)]
            v_sb = cp.tile((128, MC, HPC * 65), BF16)
            nc.vector.memset(v_sb[:], 1.0)
            ot_sb = [cp.tile((128, L), BF16, name=f"ot{p}") for p in range(2)]

            # X^T loads: lb-major so Q(lb0)/K(lb0) unblock after 16 DMAs
            for lb in range(NLB):
                for ko in range(KO):
                    nc.sync.dma_start(
                        xt_sb[:, ko, lb * LB:(lb + 1) * LB],
                        xt_d[ko * 128:(ko + 1) * 128, lb * LB:(lb + 1) * LB],
                    )

            def proj_qk(w_sb, b_sb, dst, lb):
                """(j, l) layout projection with per-partition bias via DVE."""
                for jc in range(2):
                    ps = accps.tile((128, LB), F32, name="acc_ps")
                    for ko in range(KO):
                        nc.tensor.matmul(
                            ps[:],
                            w_sb[:, ko, jc * 128:(jc + 1) * 128],
                            xt_sb[:, ko, lb * LB:(lb + 1) * LB],
                            start=(ko == 0), stop=(ko == KO - 1),
                        )
                    nc.vector.tensor_scalar(
                        dst[jc][:, lb * LB:(lb + 1) * LB], ps[:],
                        b_sb[:, jc:jc + 1], None, ADD,
                    )

            def proj_v(lb):
                """V in (l, j) layout; bias via K=1 ones-row matmul."""
                for lt in range(4):
                    vp = accps.tile((128, LB), F32, name="acc_ps")
                    for ko in range(KO):
                        nc.tensor.matmul(
                            vp[:, 0:JC],
                            xt_sb[:, ko, lb * LB + lt * 128:lb * LB + (lt + 1) * 128],
                            wv_sb[:, ko, :],
                            start=(ko == 0), stop=False,
                        )
                    nc.tensor.matmul(
                        vp[:, 0:JC], ones_sb[0:1, :], bv_sb[0:1, :],
                        start=False, stop=True,
                    )
                    nc.vector.tensor_copy(
                        v_sb[:, lb * 4 + lt, :].rearrange(
                            "p (h e) -> p h e", h=HPC)[:, :, 0:DH],
                        vp[:, 0:JC].rearrange("p (h d) -> p h d", h=HPC))

            def attn_pair_start():
                ava = accps.tile((128, LB), F32, name="acc_ps")
                avb = accps.tile((128, LB), F32, name="acc_ps")
                return ava, avb

            def attn_chunk(lb, p, m, ava, avb):
                lsl = slice(lb * LB, (lb + 1) * LB)
                msl = slice(m * 128, (m + 1) * 128)
                sc = scps.tile((128, 2 * LB), F32, name="sc_ps")
                nc.tensor.matmul(
                    sc[:, 0:LB],
                    kt_sb[p][0:64, msl], qt_sb[p][0:64, lsl],
                )
                nc.tensor.matmul(
                    sc[:, LB:2 * LB],
                    kt_sb[p][64:128, msl], qt_sb[p][64:128, lsl],
                )
                e = epool.tile((128, 2 * LB), BF16, name="e_sb")
                nc.scalar.activation(e[:], sc[:], Exp)
                # A·V per head, [V_h | 1] lhsT: row 64 = denominator
                nc.tensor.matmul(
                    ava[0:65, :],
                    v_sb[:, m, (2 * p) * 65:(2 * p) * 65 + 65],
                    e[:, 0:LB],
                    start=(m == 0), stop=(m == MC - 1),
                )
                nc.tensor.matmul(
                    avb[0:65, :],
                    v_sb[:, m, (2 * p + 1) * 65:(2 * p + 1) * 65 + 65],
                    e[:, LB:2 * LB],
                    start=(m == 0), stop=(m == MC - 1),
                )

            def attn_pair_finish(lb, p, ava, avb):
                # copy raw A·V (+denominator row) to SBUF, freeing the
                # PSUM accumulators immediately; normalize from SBUF
                lsl = slice(lb * LB, (lb + 1) * LB)
                sva = normp.tile((65, LB), F32, name="sva_sb")
                svb = normp.tile((65, LB), F32, name="svb_sb")
                nc.vector.tensor_copy(sva[:], ava[0:65, :])
                nc.vector.tensor_copy(svb[:], avb[0:65, :])
                ra = normp.tile((1, LB), F32, name="ra_sb")
                rb = normp.tile((1, LB), F32, name="rb_sb")
                nc.vector.reciprocal(ra[:], sva[64:65, :])
                nc.vector.reciprocal(rb[:], svb[64:65, :])
                rba = normp.tile((64, LB), F32, name="rba_sb")
                rbb = normp.tile((64, LB), F32, name="rbb_sb")
                nc.gpsimd.partition_broadcast(rba[:], ra[:])
                nc.gpsimd.partition_broadcast(rbb[:], rb[:])
                nc.vector.tensor_tensor(
                    ot_sb[p][0:64, lsl], sva[0:64, :], rba[:], MULT)
                nc.vector.tensor_tensor(
                    ot_sb[p][64:128, lsl], svb[0:64, :], rbb[:], MULT)

            def outproj_tile(lb, lt, ns):
                row0 = lb * LB + lt * 128
                yp = accps.tile((128, 512), F32, name="acc_ps")
                for jc in range(2):
                    nc.tensor.matmul(
                        yp[:],
                        ot_sb[jc][:, row0:row0 + 128],
                        wo_sb[:, jc, ns * 512:(ns + 1) * 512],
                        start=(jc == 0), stop=(jc == 1),
                    )
                ty = ypool.tile((128, 512), F32, name="y_sb")
                nc.any.tensor_copy(ty[:], yp[:])
                nc.sync.dma_start(
                    y_d[row0:row0 + 128, ns * 512:(ns + 1) * 512], ty[:])

            def qk_group(w_sb, b_sb, dst, lb, jc):
                ps = accps.tile((128, LB), F32, name="acc_ps")
                for ko in range(KO):
                    nc.tensor.matmul(
                        ps[:],
                        w_sb[:, ko, jc * 128:(jc + 1) * 128],
                        xt_sb[:, ko, lb * LB:(lb + 1) * LB],
                        start=(ko == 0), stop=(ko == KO - 1),
                    )
                nc.vector.tensor_scalar(
                    dst[jc][:, lb * LB:(lb + 1) * LB], ps[:],
                    b_sb[:, jc:jc + 1], None, ADD,
                )

            # Emission schedule (PE is in-order; exp is slower per chunk
            # than its paired matmuls, so non-attention matmul groups are
            # interleaved as filler to keep PE dense):
            #   Q(lb0); then lb0-pair0's m-chunks fused behind their K/V
            #   blocks; then the remaining (lb, pair) loops with
            #   out-projection(lb-1) and Q(lb+1) as filler.
            proj_qk(wq_sb, bq_sb, qt_sb, 0)
            ava0, avb0 = attn_pair_start()
            for mb in range(NLB):
                qk_group(wk_sb, bk_sb, kt_sb, mb, 0)
                qk_group(wk_sb, bk_sb, kt_sb, mb, 1)
                proj_v(mb)
                for m in range(4 * mb, 4 * mb + 4):
                    attn_chunk(0, 0, m, ava0, avb0)
            attn_pair_finish(0, 0, ava0, avb0)

            filler = []
            for lb in range(NLB):
                for p in range(2):
                    if lb == 0 and p == 0:
                        continue
                    if p == 0:
                        filler += [
                            (lambda lb=lb, lt=lt, ns=ns:
                             outproj_tile(lb - 1, lt, ns))
                            for lt in range(4) for ns in range(4)
                        ]
                    if p == 1 and lb + 1 < NLB:
                        filler += [
                            (lambda lb=lb, jc=jc:
                             qk_group(wq_sb, bq_sb, qt_sb, lb + 1, jc))
                            for jc in range(2)
                        ]
                    ava, avb = attn_pair_start()
                    for m in range(MC):
                        attn_chunk(lb, p, m, ava, avb)
                        if filler:
                            filler.pop(0)()
                    attn_pair_finish(lb, p, ava, avb)
            for f in filler:
                f()
            for lt in range(4):
                for ns in range(4):
                    outproj_tile(NLB - 1, lt, ns)

    nc.compile()
    return nc


def make_core_inputs(X, Wq_w, Wq_b, Wk_w, Wk_b, Wv_w, Wv_b, Wo_w):
    """Host-side sharding: per-core input dicts (shared XT + per-core weights)."""
    X = np.asarray(X, np.float32)
    bf = ml_dtypes.bfloat16
    xt = np.ascontiguousarray(X.T).astype(bf)
    scale = 1.0 / np.sqrt(np.float32(D))
    in_maps = []
    for c in range(NCORES):
        idx = np.array([d * NH + h for h in range(c * HPC, (c + 1) * HPC)
                        for d in range(DH)], np.int64)

        def kxj(w, s=1.0):
            # (D_in=K, JC) -> (128, KO, JC) with [p, ko, j] = w.T[ko*128+p, j]
            wt = np.ascontiguousarray((np.asarray(w, np.float32)[idx, :] * s).T)
            return np.ascontiguousarray(
                wt.reshape(KO, 128, JC).transpose(1, 0, 2)).astype(bf)

        wo = np.ascontiguousarray(np.asarray(Wo_w, np.float32)[:, idx].T)  # (JC, D)
        wo = np.ascontiguousarray(wo.reshape(2, 128, D).transpose(1, 0, 2)).astype(bf)

        def bcol(b, s=1.0):
            return np.ascontiguousarray(
                (np.asarray(b, np.float32)[idx] * s).reshape(2, 128).T)

        in_maps.append({
            "XT": xt,
            "WQ": kxj(Wq_w, scale), "WK": kxj(Wk_w), "WV": kxj(Wv_w),
            "WO": wo,
            "BQ": bcol(Wq_b, scale), "BK": bcol(Wk_b),
            "BV": np.asarray(Wv_b, np.float32)[idx].reshape(1, JC).astype(bf),
            "ONES": np.ones((1, 128), bf),
        })
    return in_maps


_prog_cache = {}


def kernel(X, Wq_w, Wq_b, Wk_w, Wk_b, Wv_w, Wv_b, Wo_w, Wo_b, _trace=False):
    from concourse.bass_utils import run_bass_kernel_spmd

    if "nc" not in _prog_cache:
        _prog_cache["nc"] = build_program()
    nc = _prog_cache["nc"]
    in_maps = make_core_inputs(X, Wq_w, Wq_b, Wk_w, Wk_b, Wv_w, Wv_b, Wo_w)
    res = run_bass_kernel_spmd(nc, in_maps, core_ids=list(range(NCORES)),
                               trace=_trace)
    y = np.zeros((L, D), np.float64)
    for r in res.results:
        y += r["Y"].astype(np.float64)
    y += np.asarray(Wo_b, np.float32).astype(np.float64)
    out = y.astype(np.float32)
    if _trace:
        kernel.last_results = res
    return out


# revision 24
# speedup vs baseline: 1.6024x; 1.0298x over previous
"""Trainium2 Bass kernel for nn_MultiHeadAttention_62319975465542.

Tensor-parallel over heads (Megatron-style): 32 heads sharded 4-per-core
across 8 NeuronCores. Each core computes its heads' QKV projections,
attention, and a partial output projection; the host sums the 8 partials
(the all-reduce after Wo) and adds the output bias.

Reference layout note: Q = (X @ Wq.T + b).reshape(L, D_HEAD, NUM_HEADS),
so head h owns interleaved feature columns {d*32 + h : d in 0..63}. The
host pre-gathers those columns into contiguous per-core blocks.

v3: all matmul inputs in bf16 (fast weight load, halved DMA), X^T fully
SBUF-resident, and emission ordered so ScalarE exp for one l-block
overlaps TensorE work of the next (Q(lb0) first, then K/V, then
attention blocks). Softmax denominators ride in the A-V matmul as a
17th "ones" V-column ([V_h | 1], M=65); normalization is reciprocal +
gpsimd partition-broadcast + one vector multiply. PSUM stays fp32.
"""

import numpy as np
import ml_dtypes

import concourse.bass as bass
import concourse.tile as tile
import concourse.mybir as mybir
from concourse import bacc

F32 = mybir.dt.float32
BF16 = mybir.dt.bfloat16
Identity = mybir.ActivationFunctionType.Identity
Exp = mybir.ActivationFunctionType.Exp
MULT = mybir.AluOpType.mult
ADD = mybir.AluOpType.add

L = 2048          # sequence length
D = 2048          # d_model
NH = 32           # total heads
DH = 64           # head dim
NCORES = 8
HPC = NH // NCORES   # heads per core = 4
JC = HPC * DH        # per-core projected width = 256
LB = 512             # l-block width
NLB = L // LB        # 4
KO = D // 128        # 16 contraction chunks
MC = L // 128        # 16 key chunks


def build_program():
    nc = bacc.Bacc("TRN2", target_bir_lowering=False, debug=False)

    xt_d = nc.dram_tensor("XT", (D, L), BF16, kind="ExternalInput")
    wq_d = nc.dram_tensor("WQ", (128, KO, JC), BF16, kind="ExternalInput")
    wk_d = nc.dram_tensor("WK", (128, KO, JC), BF16, kind="ExternalInput")
    wv_d = nc.dram_tensor("WV", (128, KO, JC), BF16, kind="ExternalInput")
    wo_d = nc.dram_tensor("WO", (128, 2, D), BF16, kind="ExternalInput")
    bq_d = nc.dram_tensor("BQ", (128, 2), F32, kind="ExternalInput")
    bk_d = nc.dram_tensor("BK", (128, 2), F32, kind="ExternalInput")
    bv_d = nc.dram_tensor("BV", (1, JC), BF16, kind="ExternalInput")
    ones_d = nc.dram_tensor("ONES", (1, 128), BF16, kind="ExternalInput")
    y_d = nc.dram_tensor("Y", (L, D), F32, kind="ExternalOutput")

    with tile.TileContext(nc) as tc, nc.allow_low_precision(
            reason="bf16 activations are within tolerance for this op"):
        with (
            tc.tile_pool(name="const", bufs=1) as cp,
            tc.tile_pool(name="epool", bufs=4) as epool,
            tc.tile_pool(name="norm", bufs=2) as normp,
            tc.tile_pool(name="ysb", bufs=3) as ypool,
            tc.tile_pool(name="scps", bufs=2, space="PSUM") as scps,
            tc.tile_pool(name="accps", bufs=4, space="PSUM") as accps,
        ):
            wq_sb = cp.tile((128, KO, JC), BF16)
            wk_sb = cp.tile((128, KO, JC), BF16)
            wv_sb = cp.tile((128, KO, JC), BF16)
            wo_sb = cp.tile((128, 2, D), BF16)
            bq_sb = cp.tile((128, 2), F32)
            bk_sb = cp.tile((128, 2), F32)
            bv_sb = cp.tile((1, JC), BF16)
            ones_sb = cp.tile((1, 128), BF16)
            xt_sb = cp.tile((128, KO, L), BF16)
            nc.sync.dma_start(wq_sb[:], wq_d[:])
            nc.sync.dma_start(wk_sb[:], wk_d[:])
            nc.sync.dma_start(wv_sb[:], wv_d[:])
            nc.sync.dma_start(wo_sb[:], wo_d[:])
            nc.sync.dma_start(bq_sb[:], bq_d[:])
            nc.sync.dma_start(bk_sb[:], bk_d[:])
            nc.sync.dma_start(bv_sb[:], bv_d[:])
            nc.sync.dma_start(ones_sb[:], ones_d[:])

            qt_sb = [cp.tile((128, L), BF16, name=f"qt{p}") for p in range(2)]
            kt_sb = [cp.tile((128, L), BF16, name=f"kt{p}") for p in range(2)]
            v_sb = cp.tile((128, MC, HPC * 65), BF16)
            nc.vector.memset(v_sb[:], 1.0)
            ot_sb = [cp.tile((128, L), BF16, name=f"ot{p}") for p in range(2)]

            # X^T loads: lb-major so Q(lb0)/K(lb0) unblock after 16 DMAs
            for lb in range(NLB):
                for ko in range(KO):
                    nc.sync.dma_start(
                        xt_sb[:, ko, lb * LB:(lb + 1) * LB],
                        xt_d[ko * 128:(ko + 1) * 128, lb * LB:(lb + 1) * LB],
                    )

            def proj_qk(w_sb, b_sb, dst, lb):
                """(j, l) layout projection with per-partition bias via DVE."""
                for jc in range(2):
                    ps = accps.tile((128, LB), F32, name="acc_ps")
                    for ko in range(KO):
                        nc.tensor.matmul(
                            ps[:],
                            w_sb[:, ko, jc * 128:(jc + 1) * 128],
                            xt_sb[:, ko, lb * LB:(lb + 1) * LB],
                            start=(ko == 0), stop=(ko == KO - 1),
                        )
                    nc.vector.tensor_scalar(
                        dst[jc][:, lb * LB:(lb + 1) * LB], ps[:],
                        b_sb[:, jc:jc + 1], None, ADD,
                    )

            def proj_v(lb):
                """V in (l, j) layout; bias via K=1 ones-row matmul."""
                for lt in range(4):
                    vp = accps.tile((128, LB), F32, name="acc_ps")
                    for ko in range(KO):
                        nc.tensor.matmul(
                            vp[:, 0:JC],
                            xt_sb[:, ko, lb * LB + lt * 128:lb * LB + (lt + 1) * 128],
                            wv_sb[:, ko, :],
                            start=(ko == 0), stop=False,
                        )
                    nc.tensor.matmul(
                        vp[:, 0:JC], ones_sb[0:1, :], bv_sb[0:1, :],
                        start=False, stop=True,
                    )
                    nc.vector.tensor_copy(
                        v_sb[:, lb * 4 + lt, :].rearrange(
                            "p (h e) -> p h e", h=HPC)[:, :, 0:DH],
                        vp[:, 0:JC].rearrange("p (h d) -> p h d", h=HPC))

            def attn_pair_start():
                ava = accps.tile((128, LB), F32, name="acc_ps")
                avb = accps.tile((128, LB), F32, name="acc_ps")
                return ava, avb

            def attn_chunk(lb, p, m, ava, avb):
                lsl = slice(lb * LB, (lb + 1) * LB)
                msl = slice(m * 128, (m + 1) * 128)
                sc = scps.tile((128, 2 * LB), F32, name="sc_ps")
                nc.tensor.matmul(
                    sc[:, 0:LB],
                    kt_sb[p][0:64, msl], qt_sb[p][0:64, lsl],
                )
                nc.tensor.matmul(
                    sc[:, LB:2 * LB],
                    kt_sb[p][64:128, msl], qt_sb[p][64:128, lsl],
                )
                e = epool.tile((128, 2 * LB), BF16, name="e_sb")
                nc.scalar.activation(e[:], sc[:], Exp)
                # A·V per head, [V_h | 1] lhsT: row 64 = denominator
                nc.tensor.matmul(
                    ava[0:65, :],
                    v_sb[:, m, (2 * p) * 65:(2 * p) * 65 + 65],
                    e[:, 0:LB],
                    start=(m == 0), stop=(m == MC - 1),
                )
                nc.tensor.matmul(
                    avb[0:65, :],
                    v_sb[:, m, (2 * p + 1) * 65:(2 * p + 1) * 65 + 65],
                    e[:, LB:2 * LB],
                    start=(m == 0), stop=(m == MC - 1),
                )

            def attn_pair_finish(lb, p, ava, avb):
                # copy raw A·V (+denominator row) to SBUF, freeing the
                # PSUM accumulators immediately; normalize from SBUF
                lsl = slice(lb * LB, (lb + 1) * LB)
                sva = normp.tile((65, LB), F32, name="sva_sb")
                svb = normp.tile((65, LB), F32, name="svb_sb")
                nc.vector.tensor_copy(sva[:], ava[0:65, :])
                nc.vector.tensor_copy(svb[:], avb[0:65, :])
                ra = normp.tile((1, LB), F32, name="ra_sb")
                rb = normp.tile((1, LB), F32, name="rb_sb")
                nc.vector.reciprocal(ra[:], sva[64:65, :])
                nc.vector.reciprocal(rb[:], svb[64:65, :])
                rba = normp.tile((64, LB), F32, name="rba_sb")
                rbb = normp.tile((64, LB), F32, name="rbb_sb")
                nc.gpsimd.partition_broadcast(rba[:], ra[:])
                nc.gpsimd.partition_broadcast(rbb[:], rb[:])
                nc.vector.tensor_tensor(
                    ot_sb[p][0:64, lsl], sva[0:64, :], rba[:], MULT)
                nc.vector.tensor_tensor(
                    ot_sb[p][64:128, lsl], svb[0:64, :], rbb[:], MULT)

            def outproj_tile(lb, lt, ns):
                row0 = lb * LB + lt * 128
                yp = accps.tile((128, 512), F32, name="acc_ps")
                for jc in range(2):
                    nc.tensor.matmul(
                        yp[:],
                        ot_sb[jc][:, row0:row0 + 128],
                        wo_sb[:, jc, ns * 512:(ns + 1) * 512],
                        start=(jc == 0), stop=(jc == 1),
                    )
                ty = ypool.tile((128, 512), F32, name="y_sb")
                nc.any.tensor_copy(ty[:], yp[:])
                nc.sync.dma_start(
                    y_d[row0:row0 + 128, ns * 512:(ns + 1) * 512], ty[:])

            def qk_group(w_sb, b_sb, dst, lb, jc):
                ps = accps.tile((128, LB), F32, name="acc_ps")
                for ko in range(KO):
                    nc.tensor.matmul(
                        ps[:],
                        w_sb[:, ko, jc * 128:(jc + 1) * 128],
                        xt_sb[:, ko, lb * LB:(lb + 1) * LB],
                        start=(ko == 0), stop=(ko == KO - 1),
                    )
                nc.vector.tensor_scalar(
                    dst[jc][:, lb * LB:(lb + 1) * LB], ps[:],
                    b_sb[:, jc:jc + 1], None, ADD,
                )

            # Emission schedule (PE is in-order; exp is slower per chunk
            # than its paired matmuls, so non-attention matmul groups are
            # interleaved as filler to keep PE dense):
            #   Q(lb0); then lb0-pair0's m-chunks fused behind their K/V
            #   blocks; then the remaining (lb, pair) loops with
            #   out-projection(lb-1) and Q(lb+1) as filler.
            proj_qk(wq_sb, bq_sb, qt_sb, 0)
            ava0, avb0 = attn_pair_start()
            for mb in range(NLB):
                qk_group(wk_sb, bk_sb, kt_sb, mb, 0)
                qk_group(wk_sb, bk_sb, kt_sb, mb, 1)
                proj_v(mb)
                for m in range(4 * mb, 4 * mb + 4):
                    attn_chunk(0, 0, m, ava0, avb0)
            attn_pair_finish(0, 0, ava0, avb0)

            filler = []
            for lb in range(NLB):
                for p in range(2):
                    if lb == 0 and p == 0:
                        continue
                    # Q(lb+1) fillers depend only on X^T/WQ (always ready)
                    # and soak the exp backlog right at a pair start; the
                    # out-projection of lb-1 waits for ot(lb-1), which is
                    # safely done one pair later.
                    if p == 0 and lb >= 1 and lb + 1 < NLB:
                        filler += [
                            (lambda lb=lb, jc=jc:
                             qk_group(wq_sb, bq_sb, qt_sb, lb + 1, jc))
                            for jc in range(2)
                        ]
                    if p == 1 and lb == 0:
                        filler += [
                            (lambda jc=jc:
                             qk_group(wq_sb, bq_sb, qt_sb, 1, jc))
                            for jc in range(2)
                        ]
                    if p == 1 and lb >= 1:
                        filler += [
                            (lambda lb=lb, lt=lt, ns=ns:
                             outproj_tile(lb - 1, lt, ns))
                            for lt in range(4) for ns in range(4)
                        ]
                    ava, avb = attn_pair_start()
                    for m in range(MC):
                        if m == 0 and filler:
                            filler.pop(0)()
                        attn_chunk(lb, p, m, ava, avb)
                        if filler:
                            filler.pop(0)()
                    attn_pair_finish(lb, p, ava, avb)
            for f in filler:
                f()
            for lt in range(4):
                for ns in range(4):
                    outproj_tile(NLB - 1, lt, ns)

    nc.compile()
    return nc


def make_core_inputs(X, Wq_w, Wq_b, Wk_w, Wk_b, Wv_w, Wv_b, Wo_w):
    """Host-side sharding: per-core input dicts (shared XT + per-core weights)."""
    X = np.asarray(X, np.float32)
    bf = ml_dtypes.bfloat16
    xt = np.ascontiguousarray(X.T).astype(bf)
    scale = 1.0 / np.sqrt(np.float32(D))
    in_maps = []
    for c in range(NCORES):
        idx = np.array([d * NH + h for h in range(c * HPC, (c + 1) * HPC)
                        for d in range(DH)], np.int64)

        def kxj(w, s=1.0):
            # (D_in=K, JC) -> (128, KO, JC) with [p, ko, j] = w.T[ko*128+p, j]
            wt = np.ascontiguousarray((np.asarray(w, np.float32)[idx, :] * s).T)
            return np.ascontiguousarray(
                wt.reshape(KO, 128, JC).transpose(1, 0, 2)).astype(bf)

        wo = np.ascontiguousarray(np.asarray(Wo_w, np.float32)[:, idx].T)  # (JC, D)
        wo = np.ascontiguousarray(wo.reshape(2, 128, D).transpose(1, 0, 2)).astype(bf)

        def bcol(b, s=1.0):
            return np.ascontiguousarray(
                (np.asarray(b, np.float32)[idx] * s).reshape(2, 128).T)

        in_maps.append({
            "XT": xt,
            "WQ": kxj(Wq_w, scale), "WK": kxj(Wk_w), "WV": kxj(Wv_w),
            "WO": wo,
            "BQ": bcol(Wq_b, scale), "BK": bcol(Wk_b),
            "BV": np.asarray(Wv_b, np.float32)[idx].reshape(1, JC).astype(bf),
            "ONES": np.ones((1, 128), bf),
        })
    return in_maps


_prog_cache = {}


def kernel(X, Wq_w, Wq_b, Wk_w, Wk_b, Wv_w, Wv_b, Wo_w, Wo_b, _trace=False):
    from concourse.bass_utils import run_bass_kernel_spmd

    if "nc" not in _prog_cache:
        _prog_cache["nc"] = build_program()
    nc = _prog_cache["nc"]
    in_maps = make_core_inputs(X, Wq_w, Wq_b, Wk_w, Wk_b, Wv_w, Wv_b, Wo_w)
    res = run_bass_kernel_spmd(nc, in_maps, core_ids=list(range(NCORES)),
                               trace=_trace)
    y = np.zeros((L, D), np.float64)
    for r in res.results:
        y += r["Y"].astype(np.float64)
    y += np.asarray(Wo_b, np.float32).astype(np.float64)
    out = y.astype(np.float32)
    if _trace:
        kernel.last_results = res
    return out
